# revision 7
# baseline (speedup 1.0000x reference)
"""Single-head causal attention on 8 TRN2 NeuronCores (Bass/Tile).

Problem: x [4, 2048, 1024] fp32; wq/wk/wv [1024, 128]; wo [128, 1024].
out = softmax_causal((x@wq)(x@wk)^T / sqrt(128)) @ (x@wv) @ wo

Sharding: 8 cores = 4 batches x 2 query-interleavings. The two cores of a
batch split the 16 query blocks (128 rows each) in a causal-load-balanced
"zebra" pattern: within each group of 4 blocks, the even core takes blocks
{4g, 4g+3}, the odd core {4g+1, 4g+2}. Each core's x arrives transposed
and column-permuted so that, per 512-column group g, its own 2 query blocks
come first. Slot j (256 queries) attends to permuted key prefix
[0 : 512*(j+1)] with a single static [512, 256] additive mask handling the
diagonal group (the mask data differs between even/odd cores; the program
is identical -> single SPMD NEFF).

On-device layout (per core):
  xt   [1024 d, 2048 s]   (transposed, permuted x)    -> SBUF [128, 8, 2048]
  QT/KT/VT [128 h, s]     via matmul(lhsT=w_chunk, rhs=xt_chunk), fp32r
  V    [s, 128 h]         via PE transpose of VT blocks
  ST   [k, 256 q] psum    via matmul(lhsT=KT_block, rhs=QT_slot)
  PT = exp(scale*(ST+mask))  (ACT, no max subtraction: |score| <= ~12)
  den  [1, 256]  psum     via matmul(lhsT=ones[128,1], rhs=PT_block) accum
  ctxT [128 h, 256 q]     via matmul(lhsT=V_block, rhs=PT_block) accum
  out  [q, 1024]          via matmul(lhsT=ctxT_qblock, rhs=wo), scaled by
                          1/den per query row (tensor_scalar_mul)

All matmuls use float32r (FP22 multiply, fp32 accumulate): 1 cycle/row at
free dim >= 256 (4x faster than fp32) with ~2^-14 relative precision.
"""

import numpy as np

import concourse.bass as bass
from concourse import bacc
import concourse.mybir as mybir
import concourse.tile as tile
from concourse.bass_utils import run_bass_kernel_spmd
from concourse.masks import make_identity

F32 = mybir.dt.float32
F32R = mybir.dt.float32r

D_MODEL = 1024
D_HEAD = 128
SEQ = 2048
BATCH = 4
NCORES = 8
P = 128           # partitions / block size
DC = D_MODEL // P  # 8 d_model chunks
NB = SEQ // P      # 16 seq blocks
NSLOT = 4          # query slots per core
QW = 256           # queries per slot
NQ = NSLOT * QW    # 1024 queries per core
SCALE = 1.0 / float(np.sqrt(D_HEAD))
MASK_NEG = -1e9


def block_order(parity: int) -> list[int]:
    order = []
    for g in range(4):
        if parity == 0:
            order += [4 * g, 4 * g + 3, 4 * g + 1, 4 * g + 2]
        else:
            order += [4 * g + 1, 4 * g + 2, 4 * g, 4 * g + 3]
    return order


def make_maskT(parity: int) -> np.ndarray:
    """Additive mask for the diagonal 512-key group, transposed: [512 k, 256 q]."""
    P4 = block_order(parity)[:4]
    m = np.zeros((512, 256), dtype=np.float32)
    kr = np.arange(P)[:, None]
    qc = np.arange(P)[None, :]
    tri = np.where(kr <= qc, 0.0, MASK_NEG).astype(np.float32)
    for kb2 in range(4):
        K = P4[kb2]
        for qb2 in range(2):
            Q = P4[qb2]
            blk = m[P * kb2:P * (kb2 + 1), P * qb2:P * (qb2 + 1)]
            if K < Q:
                blk[:] = 0.0
            elif K > Q:
                blk[:] = MASK_NEG
            else:
                blk[:] = tri
    return m


def _attention_kernel(tc: tile.TileContext, xt_d, wq_d, wk_d, wv_d, wo_d,
                      maskt_d, out_d):
    nc = tc.nc

    with (
        tc.tile_pool(name="const", bufs=1) as const_pool,
        tc.tile_pool(name="big", bufs=1) as big_pool,
        tc.tile_pool(name="ptp", bufs=2) as pt_pool,
        tc.tile_pool(name="outp", bufs=3) as out_pool,
    ):
        # ---- constants / weights ----
        ident = const_pool.tile([P, P], F32)
        make_identity(nc, ident)
        ones_f = const_pool.tile([P, 1], F32)
        nc.vector.memset(ones_f, 1.0)
        ones = const_pool.tile([P, 1], F32R)
        nc.vector.tensor_copy(ones, ones_f)

        wq_sb = const_pool.tile([P, DC, P], F32R)
        nc.sync.dma_start(out=wq_sb, in_=wq_d.rearrange("(c p) h -> p c h", p=P))
        wk_sb = const_pool.tile([P, DC, P], F32R)
        nc.sync.dma_start(out=wk_sb, in_=wk_d.rearrange("(c p) h -> p c h", p=P))
        wv_sb = const_pool.tile([P, DC, P], F32R)
        nc.sync.dma_start(out=wv_sb, in_=wv_d.rearrange("(c p) h -> p c h", p=P))
        wo_sb = const_pool.tile([P, D_MODEL], F32R)
        nc.sync.dma_start(out=wo_sb, in_=wo_d)
        maskt_sb = const_pool.tile([P, 4, QW], F32)
        nc.sync.dma_start(out=maskt_sb,
                          in_=maskt_d.rearrange("(b p) q -> p b q", p=P))

        # ---- x (transposed, permuted): 8 d-chunk tiles [128, 2048] ----
        # (one tile + one DMA per chunk: consumers then wait on exactly one
        # DMA-queue semaphore each, avoiding the per-instruction wait limit)
        xt_sb = []
        for c in range(DC):
            t = big_pool.tile([P, SEQ], F32R, name=f"xt_sb{c}")
            nc.sync.dma_start(out=t, in_=xt_d[P * c:P * (c + 1), :])
            xt_sb.append(t)

        qt_sb = big_pool.tile([P, NQ], F32R)
        kt_sb = big_pool.tile([P, SEQ], F32R)
        vt_sb = big_pool.tile([P, SEQ], F32)
        v_sb = big_pool.tile([P, SEQ], F32R)   # normal-layout V, block kb at cols [128kb:..)
        ctxt_sb = big_pool.tile([P, NQ], F32R)
        den_sb = big_pool.tile([1, NQ], F32)

        # ---- phase B: projections ----
        with tc.tile_pool(name="pj_ps", bufs=2, space="PSUM") as pj:
            # QT per slot: rhs columns [512j : 512j+256] of xt
            for j in range(NSLOT):
                ps = pj.tile([P, 512], F32, tag="pj")
                for c in range(DC):
                    nc.tensor.matmul(
                        ps[:, 0:QW],
                        lhsT=wq_sb[:, c, :],
                        rhs=xt_sb[c][:, 512 * j:512 * j + QW],
                        start=(c == 0), stop=(c == DC - 1))
                nc.vector.tensor_copy(qt_sb[:, QW * j:QW * (j + 1)], ps[:, 0:QW])
            # KT / VT in 512-wide tiles
            for t in range(4):
                ps = pj.tile([P, 512], F32, tag="pj")
                for c in range(DC):
                    nc.tensor.matmul(
                        ps,
                        lhsT=wk_sb[:, c, :],
                        rhs=xt_sb[c][:, 512 * t:512 * (t + 1)],
                        start=(c == 0), stop=(c == DC - 1))
                nc.vector.tensor_copy(kt_sb[:, 512 * t:512 * (t + 1)], ps)
            for t in range(4):
                ps = pj.tile([P, 512], F32, tag="pj")
                for c in range(DC):
                    nc.tensor.matmul(
                        ps,
                        lhsT=wv_sb[:, c, :],
                        rhs=xt_sb[c][:, 512 * t:512 * (t + 1)],
                        start=(c == 0), stop=(c == DC - 1))
                nc.vector.tensor_copy(vt_sb[:, 512 * t:512 * (t + 1)], ps)
            # V normal layout via PE transpose of VT blocks
            for kb in range(NB):
                pt = pj.tile([P, P], F32, tag="tr")
                nc.tensor.transpose(pt, vt_sb[:, P * kb:P * (kb + 1)], ident)
                nc.vector.tensor_copy(v_sb[:, P * kb:P * (kb + 1)], pt)

        # ---- phase C: attention per slot ----
        with tc.tile_pool(name="att_ps", bufs=2, space="PSUM") as att_ps:
            for j in range(NSLOT):
                nkb = 4 * (j + 1)
                qt_j = qt_sb[:, QW * j:QW * (j + 1)]
                pt_slab = pt_pool.tile([P, 4 * NSLOT * QW], F32R, tag="pt")
                den_ps = att_ps.tile([1, QW], F32, tag="den")
                ctx_ps = att_ps.tile([P, QW], F32, tag="ctx")
                for g in range(j + 1):
                    st_ps = att_ps.tile([P, 4 * QW], F32, tag="st")
                    for k2 in range(4):
                        kb = 4 * g + k2
                        nc.tensor.matmul(
                            st_ps[:, QW * k2:QW * (k2 + 1)],
                            lhsT=kt_sb[:, P * kb:P * (kb + 1)],
                            rhs=qt_j,
                            start=True, stop=True)
                    if g == j:
                        for k2 in range(4):
                            nc.vector.tensor_add(
                                st_ps[:, QW * k2:QW * (k2 + 1)],
                                st_ps[:, QW * k2:QW * (k2 + 1)],
                                maskt_sb[:, k2, :])
                    nc.scalar.activation(
                        out=pt_slab[:, 4 * QW * g:4 * QW * (g + 1)],
                        in_=st_ps,
                        func=mybir.ActivationFunctionType.Exp,
                        scale=SCALE)
                for kb in range(nkb):
                    pt_kb = pt_slab[:, QW * kb:QW * (kb + 1)]
                    nc.tensor.matmul(
                        den_ps,
                        lhsT=ones,
                        rhs=pt_kb,
                        start=(kb == 0), stop=(kb == nkb - 1))
                    nc.tensor.matmul(
                        ctx_ps,
                        lhsT=v_sb[:, P * kb:P * (kb + 1)],
                        rhs=pt_kb,
                        start=(kb == 0), stop=(kb == nkb - 1))
                nc.vector.tensor_copy(ctxt_sb[:, QW * j:QW * (j + 1)], ctx_ps)
                nc.vector.tensor_copy(den_sb[:, QW * j:QW * (j + 1)], den_ps)

            # denominators -> per-partition layout, one [128, 1] tile per
            # q-block (separate tiles: one DMA-sem wait per consumer)
            rden = []
            for qb in range(NQ // P):
                dp = big_pool.tile([P, 1], F32, name=f"denp{qb}")
                nc.sync.dma_start(out=dp,
                                  in_=den_sb[0:1, P * qb:P * (qb + 1)])
                rd = big_pool.tile([P, 1], F32, name=f"rden{qb}")
                nc.vector.reciprocal(rd, dp)
                rden.append(rd)

            # ---- phase E: output projection ----
            for qb in range(NQ // P):
                ps = att_ps.tile([P, D_MODEL], F32, tag="st")
                for t in range(2):
                    nc.tensor.matmul(
                        ps[:, 512 * t:512 * (t + 1)],
                        lhsT=ctxt_sb[:, P * qb:P * (qb + 1)],
                        rhs=wo_sb[:, 512 * t:512 * (t + 1)],
                        start=True, stop=True)
                ot = out_pool.tile([P, D_MODEL], F32, tag="ot")
                nc.vector.tensor_scalar_mul(ot, ps, rden[qb])
                nc.sync.dma_start(out=out_d[P * qb:P * (qb + 1), :], in_=ot)


_NC_CACHE = None


def build_nc() -> bass.Bass:
    global _NC_CACHE
    if _NC_CACHE is not None:
        return _NC_CACHE
    nc = bacc.Bacc("TRN2", target_bir_lowering=False, debug=False)
    xt_d = nc.dram_tensor("xt", [D_MODEL, SEQ], F32R, kind="ExternalInput").ap()
    wq_d = nc.dram_tensor("wq", [D_MODEL, D_HEAD], F32R, kind="ExternalInput").ap()
    wk_d = nc.dram_tensor("wk", [D_MODEL, D_HEAD], F32R, kind="ExternalInput").ap()
    wv_d = nc.dram_tensor("wv", [D_MODEL, D_HEAD], F32R, kind="ExternalInput").ap()
    wo_d = nc.dram_tensor("wo", [D_HEAD, D_MODEL], F32R, kind="ExternalInput").ap()
    maskt_d = nc.dram_tensor("maskt", [512, QW], F32, kind="ExternalInput").ap()
    out_d = nc.dram_tensor("out", [NQ, D_MODEL], F32, kind="ExternalOutput").ap()
    with tile.TileContext(nc) as tc:
        _attention_kernel(tc, xt_d, wq_d, wk_d, wv_d, wo_d, maskt_d, out_d)
    nc.compile()
    _NC_CACHE = nc
    return nc


def kernel(x, wq, wk, wv, wo, _trace=False, _trace_kwargs=None):
    x = np.ascontiguousarray(np.asarray(x, dtype=np.float32))
    wq = np.ascontiguousarray(np.asarray(wq, dtype=np.float32))
    wk = np.ascontiguousarray(np.asarray(wk, dtype=np.float32))
    wv = np.ascontiguousarray(np.asarray(wv, dtype=np.float32))
    wo = np.ascontiguousarray(np.asarray(wo, dtype=np.float32))

    nc = build_nc()

    masks = {p: make_maskT(p) for p in (0, 1)}
    in_maps = []
    for core in range(NCORES):
        b, parity = core // 2, core % 2
        order = block_order(parity)
        perm = np.concatenate([np.arange(P) + P * o for o in order])
        xt = np.ascontiguousarray(x[b][perm, :].T)
        in_maps.append({
            "xt": xt, "wq": wq, "wk": wk, "wv": wv, "wo": wo,
            "maskt": masks[parity],
        })

    res = run_bass_kernel_spmd(
        nc, in_maps, core_ids=list(range(NCORES)),
        trace=_trace, **(_trace_kwargs or {}))

    out = np.empty_like(x)
    for core in range(NCORES):
        b, parity = core // 2, core % 2
        order = block_order(parity)
        core_out = res.results[core]["out"]
        for j in range(NSLOT):
            for i in range(2):
                qb = order[4 * j + i]
                out[b, P * qb:P * (qb + 1), :] = \
                    core_out[QW * j + P * i:QW * j + P * (i + 1), :]
    if _trace:
        return out, res
    return out


# revision 11
# speedup vs baseline: 1.1791x; 1.1791x over previous
"""Single-head causal attention on 8 TRN2 NeuronCores (Bass/Tile).

Problem: x [4, 2048, 1024] fp32; wq/wk/wv [1024, 128]; wo [128, 1024].
out = softmax_causal((x@wq)(x@wk)^T / sqrt(128)) @ (x@wv) @ wo

Sharding: 8 cores = 4 batches x 2 query-interleavings. The two cores of a
batch split the 16 query blocks (128 rows each) in a causal-load-balanced
"zebra" pattern: within each group of 4 blocks, the even core takes blocks
{4g, 4g+3}, the odd core {4g+1, 4g+2}. Each core's x arrives transposed
and column-permuted so that, per 512-column group g, its own 2 query blocks
come first. Slot j (256 queries) attends to permuted key prefix
[0 : 512*(j+1)] with a single static [512, 256] additive mask handling the
diagonal group (the mask data differs between even/odd cores; the program
is identical -> single SPMD NEFF).

On-device layout (per core):
  xt   [1024 d, 2048 s]  bf16 (transposed, permuted x), 8 chunk tiles
  QT/KT/VT [128 h, s]    via matmul(lhsT=w_chunk, rhs=xt_chunk) -> bf16
  V    [s, 128 h]        via PE transpose of VT blocks
  ST   [k, 256 q] psum   via matmul(lhsT=KT_block, rhs=QT_slot)
  PT = exp(scale*(ST+mask))  (ACT, no max subtraction: |score| <= ~12)
  den  [1, 256]  psum    via matmul(lhsT=ones[128,1], rhs=PT_block) accum
  ctxT [128 h, 256 q]    via matmul(lhsT=V_block, rhs=PT_block) accum
  out  [q, 1024]         via matmul(lhsT=ctxT_qblock, rhs=wo), scaled by
                         1/den per query row on GpSimd

Matmuls in bf16 (1 cycle/row, FWL weight loads), accumulation fp32 in
PSUM. Projection loops run d-chunk-outer so each arriving x chunk
releases a dense burst of matmuls (keeps the PE HAM clock warm); the
attention slots are software-pipelined (scores of slot j+1 overlap
softmax-denominator/AV of slot j and output projection of slot j-1).
"""

import numpy as np
import ml_dtypes

import concourse.bass as bass
from concourse import bacc
import concourse.mybir as mybir
import concourse.tile as tile
from concourse.bass_utils import run_bass_kernel_spmd
from concourse.masks import make_identity

F32 = mybir.dt.float32
BF16 = mybir.dt.bfloat16

D_MODEL = 1024
D_HEAD = 128
SEQ = 2048
BATCH = 4
NCORES = 8
P = 128            # partitions / block size
DC = D_MODEL // P  # 8 d_model chunks
NB = SEQ // P      # 16 seq blocks
NSLOT = 4          # query slots per core
QW = 256           # queries per slot
NQ = NSLOT * QW    # 1024 queries per core
SCALE = 1.0 / float(np.sqrt(D_HEAD))
MASK_NEG = -1e9


def block_order(parity: int) -> list[int]:
    order = []
    for g in range(4):
        if parity == 0:
            order += [4 * g, 4 * g + 3, 4 * g + 1, 4 * g + 2]
        else:
            order += [4 * g + 1, 4 * g + 2, 4 * g, 4 * g + 3]
    return order


def make_maskT(parity: int) -> np.ndarray:
    """Additive mask for the diagonal 512-key group, transposed: [512 k, 256 q]."""
    P4 = block_order(parity)[:4]
    m = np.zeros((512, 256), dtype=np.float32)
    kr = np.arange(P)[:, None]
    qc = np.arange(P)[None, :]
    tri = np.where(kr <= qc, 0.0, MASK_NEG).astype(np.float32)
    for kb2 in range(4):
        K = P4[kb2]
        for qb2 in range(2):
            Q = P4[qb2]
            blk = m[P * kb2:P * (kb2 + 1), P * qb2:P * (qb2 + 1)]
            if K < Q:
                blk[:] = 0.0
            elif K > Q:
                blk[:] = MASK_NEG
            else:
                blk[:] = tri
    return m


def _attention_kernel(tc: tile.TileContext, xt_d, wq_d, wk_d, wv_d, wo_d,
                      maskt_d, out_d):
    nc = tc.nc

    with (
        tc.tile_pool(name="const", bufs=1) as const_pool,
        tc.tile_pool(name="big", bufs=1) as big_pool,
        tc.tile_pool(name="ptp", bufs=2) as pt_pool,
        tc.tile_pool(name="outp", bufs=3) as out_pool,
    ):
        # ---- weights first (small), then x chunks: first matmul burst can
        # start as soon as wq/wk/wv + chunk 0 have landed ----
        wq_sb = const_pool.tile([P, DC, P], BF16)
        nc.sync.dma_start(out=wq_sb, in_=wq_d.rearrange("(c p) h -> p c h", p=P))
        wk_sb = const_pool.tile([P, DC, P], BF16)
        nc.sync.dma_start(out=wk_sb, in_=wk_d.rearrange("(c p) h -> p c h", p=P))
        wv_sb = const_pool.tile([P, DC, P], BF16)
        nc.sync.dma_start(out=wv_sb, in_=wv_d.rearrange("(c p) h -> p c h", p=P))

        xt_sb = []
        for c in range(DC):
            t = big_pool.tile([P, SEQ], BF16, name=f"xt_sb{c}")
            nc.sync.dma_start(out=t, in_=xt_d[P * c:P * (c + 1), :])
            xt_sb.append(t)

        wo_sb = const_pool.tile([P, D_MODEL], BF16)
        nc.sync.dma_start(out=wo_sb, in_=wo_d)
        maskt_sb = const_pool.tile([P, 4, QW], F32)
        nc.sync.dma_start(out=maskt_sb,
                          in_=maskt_d.rearrange("(b p) q -> p b q", p=P))

        # ---- constants ----
        ident = const_pool.tile([P, P], BF16)
        make_identity(nc, ident)
        ones = const_pool.tile([P, 1], BF16)
        nc.vector.memset(ones, 1.0)

        qt_sb = big_pool.tile([P, NQ], BF16)
        kt_sb = big_pool.tile([P, SEQ], BF16)
        vt_sb = big_pool.tile([P, SEQ], BF16)
        v_sb = big_pool.tile([P, SEQ], BF16)  # normal-layout V, block kb at cols [128kb:)
        ctxt_sb = big_pool.tile([P, NQ], BF16)
        den_sb = big_pool.tile([1, NQ], F32)

        # ---- phase B1: QT + KT, d-chunk OUTER (dense matmul burst per
        # arriving chunk; 6 psum accumulators live: 2 QT-pair + 4 KT) ----
        with tc.tile_pool(name="pj1_ps", bufs=1, space="PSUM") as pj1:
            qt_ps = [pj1.tile([P, 512], F32, name=f"qt_ps{i}", tag=f"qt{i}")
                     for i in range(2)]
            kt_ps = [pj1.tile([P, 512], F32, name=f"kt_ps{i}", tag=f"kt{i}")
                     for i in range(4)]
            for c in range(DC):
                xr = xt_sb[c].rearrange("p (g q) -> p g q", q=QW)
                for i in range(2):
                    nc.tensor.matmul(
                        qt_ps[i],
                        lhsT=wq_sb[:, c, :],
                        rhs=xr[:, 4 * i:4 * i + 3:2, :],
                        start=(c == 0), stop=(c == DC - 1),
                        skip_group_check=True)
                for t in range(4):
                    nc.tensor.matmul(
                        kt_ps[t],
                        lhsT=wk_sb[:, c, :],
                        rhs=xt_sb[c][:, 512 * t:512 * (t + 1)],
                        start=(c == 0), stop=(c == DC - 1),
                        skip_group_check=True)
            for i in range(2):
                nc.vector.tensor_copy(qt_sb[:, 512 * i:512 * (i + 1)], qt_ps[i])
            for t in range(4):
                nc.vector.tensor_copy(kt_sb[:, 512 * t:512 * (t + 1)], kt_ps[t])

        # ---- phase B2: VT + PE-transpose to V ----
        with tc.tile_pool(name="pj2_ps", bufs=1, space="PSUM") as pj2:
            vt_ps = [pj2.tile([P, 512], F32, name=f"vt_ps{i}", tag=f"vt{i}")
                     for i in range(4)]
            for c in range(DC):
                for t in range(4):
                    nc.tensor.matmul(
                        vt_ps[t],
                        lhsT=wv_sb[:, c, :],
                        rhs=xt_sb[c][:, 512 * t:512 * (t + 1)],
                        start=(c == 0), stop=(c == DC - 1),
                        skip_group_check=True)
            for t in range(4):
                nc.vector.tensor_copy(vt_sb[:, 512 * t:512 * (t + 1)], vt_ps[t])
            for kb in range(NB):
                ptr = pj2.tile([P, P], BF16, tag="tr", bufs=2)
                nc.tensor.transpose(ptr, vt_sb[:, P * kb:P * (kb + 1)], ident)
                nc.vector.tensor_copy(v_sb[:, P * kb:P * (kb + 1)], ptr)

        # ---- phase C/E: attention, software-pipelined across slots ----
        # stage lag: scores/exp(j) || den+AV(j-1) || outproj(j-2)
        with tc.tile_pool(name="att_ps", bufs=1, space="PSUM") as att_ps:
            pt_slabs = [None] * NSLOT
            denav_ps = [None] * NSLOT

            def st_exp_stage(j):
                nkb = 4 * (j + 1)
                qt_j = qt_sb[:, QW * j:QW * (j + 1)]
                pt_slab = pt_pool.tile([P, 4 * NSLOT * QW], BF16, tag="pt")
                pt_slabs[j] = pt_slab
                for g in range(j + 1):
                    st_ps = att_ps.tile([P, 4 * QW], F32, tag="st", bufs=2)
                    for k2 in range(4):
                        kb = 4 * g + k2
                        nc.tensor.matmul(
                            st_ps[:, QW * k2:QW * (k2 + 1)],
                            lhsT=kt_sb[:, P * kb:P * (kb + 1)],
                            rhs=qt_j,
                            start=True, stop=True)
                    if g == j:
                        mflat = maskt_sb.rearrange("p b q -> p (b q)")
                        nc.vector.tensor_add(st_ps, st_ps, mflat)
                    nc.scalar.activation(
                        out=pt_slab[:, 4 * QW * g:4 * QW * (g + 1)],
                        in_=st_ps,
                        func=mybir.ActivationFunctionType.Exp,
                        scale=SCALE)

            def denav_stage(j):
                nkb = 4 * (j + 1)
                pt_slab = pt_slabs[j]
                den_ps = att_ps.tile([1, QW], F32, tag="den", bufs=2)
                ctx_ps = att_ps.tile([P, QW], F32, tag="ctx", bufs=2)
                denav_ps[j] = (den_ps, ctx_ps)
                for kb in range(nkb):
                    pt_kb = pt_slab[:, QW * kb:QW * (kb + 1)]
                    nc.tensor.matmul(
                        den_ps, lhsT=ones, rhs=pt_kb,
                        start=(kb == 0), stop=(kb == nkb - 1),
                        skip_group_check=True)
                    nc.tensor.matmul(
                        ctx_ps, lhsT=v_sb[:, P * kb:P * (kb + 1)], rhs=pt_kb,
                        start=(kb == 0), stop=(kb == nkb - 1),
                        skip_group_check=True)
                nc.vector.tensor_copy(ctxt_sb[:, QW * j:QW * (j + 1)], ctx_ps)
                nc.vector.tensor_copy(den_sb[:, QW * j:QW * (j + 1)], den_ps)

            def out_stage(j):
                for i in range(2):
                    qb = 2 * j + i
                    dp = big_pool.tile([P, 1], F32, name=f"denp{qb}")
                    nc.sync.dma_start(out=dp,
                                      in_=den_sb[0:1, P * qb:P * (qb + 1)])
                    rd = big_pool.tile([P, 1], F32, name=f"rden{qb}")
                    nc.vector.reciprocal(rd, dp)
                    # shares the "st" slots: same [128, 1024] shape, and the
                    # ST users of slot j+1 are done with a slot by the time
                    # outproj(j-1) needs one
                    ps = att_ps.tile([P, D_MODEL], F32, tag="st", bufs=2)
                    for t in range(2):
                        nc.tensor.matmul(
                            ps[:, 512 * t:512 * (t + 1)],
                            lhsT=ctxt_sb[:, P * qb:P * (qb + 1)],
                            rhs=wo_sb[:, 512 * t:512 * (t + 1)],
                            start=True, stop=True)
                    ot = out_pool.tile([P, D_MODEL], F32, tag="ot")
                    nc.vector.tensor_scalar_mul(ot, ps, rd)
                    nc.sync.dma_start(out=out_d[P * qb:P * (qb + 1), :], in_=ot)

            for j in range(NSLOT + 2):
                if j < NSLOT:
                    st_exp_stage(j)
                if 1 <= j <= NSLOT:
                    denav_stage(j - 1)
                if j >= 2:
                    out_stage(j - 2)


_NC_CACHE = None


def build_nc() -> bass.Bass:
    global _NC_CACHE
    if _NC_CACHE is not None:
        return _NC_CACHE
    nc = bacc.Bacc("TRN2", target_bir_lowering=False, debug=False)
    xt_d = nc.dram_tensor("xt", [D_MODEL, SEQ], BF16, kind="ExternalInput").ap()
    wq_d = nc.dram_tensor("wq", [D_MODEL, D_HEAD], BF16, kind="ExternalInput").ap()
    wk_d = nc.dram_tensor("wk", [D_MODEL, D_HEAD], BF16, kind="ExternalInput").ap()
    wv_d = nc.dram_tensor("wv", [D_MODEL, D_HEAD], BF16, kind="ExternalInput").ap()
    wo_d = nc.dram_tensor("wo", [D_HEAD, D_MODEL], BF16, kind="ExternalInput").ap()
    maskt_d = nc.dram_tensor("maskt", [512, QW], F32, kind="ExternalInput").ap()
    out_d = nc.dram_tensor("out", [NQ, D_MODEL], F32, kind="ExternalOutput").ap()
    with tile.TileContext(nc) as tc:
        _attention_kernel(tc, xt_d, wq_d, wk_d, wv_d, wo_d, maskt_d, out_d)
    nc.compile()
    _NC_CACHE = nc
    return nc


def kernel(x, wq, wk, wv, wo, _trace=False, _trace_kwargs=None):
    x = np.asarray(x, dtype=np.float32)
    bf = ml_dtypes.bfloat16
    wq_b = np.ascontiguousarray(np.asarray(wq, dtype=np.float32).astype(bf))
    wk_b = np.ascontiguousarray(np.asarray(wk, dtype=np.float32).astype(bf))
    wv_b = np.ascontiguousarray(np.asarray(wv, dtype=np.float32).astype(bf))
    wo_b = np.ascontiguousarray(np.asarray(wo, dtype=np.float32).astype(bf))

    nc = build_nc()

    masks = {p: make_maskT(p) for p in (0, 1)}
    in_maps = []
    for core in range(NCORES):
        b, parity = core // 2, core % 2
        order = block_order(parity)
        perm = np.concatenate([np.arange(P) + P * o for o in order])
        xt = np.ascontiguousarray(x[b][perm, :].T.astype(bf))
        in_maps.append({
            "xt": xt, "wq": wq_b, "wk": wk_b, "wv": wv_b, "wo": wo_b,
            "maskt": masks[parity],
        })

    res = run_bass_kernel_spmd(
        nc, in_maps, core_ids=list(range(NCORES)),
        trace=_trace, **(_trace_kwargs or {}))

    out = np.empty_like(x)
    for core in range(NCORES):
        b, parity = core // 2, core % 2
        order = block_order(parity)
        core_out = res.results[core]["out"]
        for j in range(NSLOT):
            for i in range(2):
                qb = order[4 * j + i]
                out[b, P * qb:P * (qb + 1), :] = \
                    core_out[QW * j + P * i:QW * j + P * (i + 1), :]
    if _trace:
        return out, res
    return out
